# revision 7
# baseline (speedup 1.0000x reference)
"""MAMDense Trainium2 kernel.

C[m, n] = max_k(x[m,k] * W[n,k]) + min_k(x[m,k] * W[n,k]) + bias[n]
x: [2048, 1024] f32, W: [1024, 1024] f32, bias: [1024] f32 -> C: [2048, 1024] f32

Strategy (data parallel over M, 8 cores, 256 rows each):
- x rows on SBUF partitions (2 tiles of 128 rows x 1024 K).
- Weight rows stored one-per-partition in SBUF ([128, 8*1024]); for each
  output column n, the PE broadcasts W[n, :] across all 128 partitions via a
  ones-vector outer product into PSUM.
- One fused custom DVE instruction per (m-tile, n) computes
  running_max(x*w) + running_min(x*w) + bias[n] over K in a single pass;
  the output AP has free-stride 0 so the final (k=K-1) value - the answer -
  lands directly in C[:, n]. No separate reduce or extract instructions.
- C tiles DMA straight out in natural [M, N] layout.
"""
import numpy as np

M, K, N = 2048, 1024, 1024
N_CORES = 8
M_LOC = M // N_CORES  # 256
P = 128
FMAX = 3.4028234663852886e38

_STATE = {}


def _register_mam_op():
    """Register the fused multiply->scan(max)+scan(min)+bias DVE op."""
    import concourse.dve_ops as dve_ops
    from concourse.dve_ops import DveOp
    from concourse.dve_spec import (
        Spec, Src0, Src1, C0, C1, scan, AluOp, lower, _has_src1,
    )
    from concourse.dve_uop import DveOpSpec

    name = "MAM_BIAS_FUSED_ANT"
    for op in dve_ops.OPS:
        if op.name == name:
            return op

    prod = Src0 * Src1

    def _ref(in0, in1, s0, s1, imm2):
        pr = in0 * in1
        return (np.maximum.accumulate(pr, axis=-1)
                + np.minimum.accumulate(pr, axis=-1) + s1)

    spec = Spec(
        body=scan(AluOp.MAX, prod) + scan(AluOp.MIN, prod, init=C0) + C1,
        reference=_ref,
    )
    shas = {}
    for ver in ("v3", "v4"):
        try:
            s = DveOpSpec(name=name, opcode=1, uops=lower(spec, ver=ver),
                          rd1_en=_has_src1(spec))
            shas[ver] = s.sha(ver)
        except Exception:
            pass
    op = DveOp(name, spec, subdim=False, uops_sha=shas)
    dve_ops.OPS.append(op)
    dve_ops._SUB_OPCODE_FOR_NAME[name] = (
        dve_ops._CUSTOM_DVE_ROW_BASE + len(dve_ops.OPS) - 1
    )
    dve_ops.CUSTOM_DVE_SPECS[name] = spec
    return op


def build_nc(replicas: int = 1):
    """Build + compile the per-core Bacc program. `replicas` repeats the
    compute body (for timing-by-differencing in test harnesses)."""
    import concourse.bacc as bacc
    import concourse.mybir as mybir
    from concourse.tile import TileContext

    MAM = _register_mam_op()

    nc = bacc.Bacc("TRN2", target_bir_lowering=False, debug=False)
    x = nc.dram_tensor("x", [M_LOC, K], mybir.dt.float32, kind="ExternalInput")
    w = nc.dram_tensor("weight", [N, K], mybir.dt.float32, kind="ExternalInput")
    b = nc.dram_tensor("bias", [N], mybir.dt.float32, kind="ExternalInput")
    out = nc.dram_tensor("out", [M_LOC, N], mybir.dt.float32,
                         kind="ExternalOutput")

    NT = M_LOC // P  # 2 m-tiles

    with TileContext(nc) as tc:
        with tc.tile_pool(name="const", bufs=1) as cpool, \
             tc.tile_pool(name="psum", bufs=4, space="PSUM") as ppool:
            # --- loads -----------------------------------------------------
            xt = []
            for t in range(NT):
                xti = cpool.tile([P, K], mybir.dt.float32, name=f"xt{t}",
                                 tag=f"xt{t}")
                nc.sync.dma_start(out=xti[:], in_=x.ap()[t * P:(t + 1) * P, :])
                xt.append(xti)
            bias_t = cpool.tile([1, N], mybir.dt.float32, tag="bias_t")
            nc.sync.dma_start(out=bias_t[:], in_=b.ap()[None, :])
            ones = cpool.tile([1, P], mybir.dt.float32, tag="ones")
            nc.gpsimd.memset(ones[:], 1.0)

            # bias broadcast across partitions: ones^T @ bias_row -> PSUM
            bias_bc = cpool.tile([P, N], mybir.dt.float32, tag="bias_bc")
            pb0 = ppool.tile([P, N], mybir.dt.float32, tag="pb")
            for h in range(N // 512):
                nc.tensor.matmul(pb0[:, h * 512:(h + 1) * 512], ones[:],
                                 bias_t[0:1, h * 512:(h + 1) * 512],
                                 start=True, stop=True)
            nc.scalar.copy(bias_bc[:], pb0[:])

            ct = [cpool.tile([P, N], mybir.dt.float32, name=f"ct{t}",
                             tag=f"ct{t}")
                  for t in range(NT)]

            # --- main loop ---------------------------------------------------
            # W rows are staged from DRAM into a partition-0 tile (GROUP rows
            # per DMA) so the PE broadcast's rhs satisfies the base-partition
            # constraint; PE then replicates each row across 128 partitions.
            GROUP = 16
            with tc.tile_pool(name="stage", bufs=2) as spool:
                for _ in range(replicas):
                    for g in range(N // GROUP):
                        st = spool.tile([1, GROUP * K], mybir.dt.float32,
                                        tag="st")
                        nc.sync.dma_start(
                            out=st[:],
                            in_=w.ap()[g * GROUP:(g + 1) * GROUP, :].rearrange(
                                "(o r) k -> o (r k)", o=1))
                        for j in range(GROUP):
                            n = g * GROUP + j
                            pb = ppool.tile([P, K], mybir.dt.float32, tag="pb")
                            for h in range(K // 512):
                                nc.tensor.matmul(
                                    pb[:, h * 512:(h + 1) * 512], ones[:],
                                    st[0:1, j * K + h * 512:j * K + (h + 1) * 512],
                                    start=True, stop=True)
                            for t in range(NT):
                                nc.vector._custom_dve(
                                    MAM,
                                    out=ct[t][:, n:n + 1].broadcast_to([P, K]),
                                    in0=xt[t][:],
                                    in1=pb[:],
                                    s0=FMAX,
                                    s1=bias_bc[:, n:n + 1],
                                )

            # --- store -------------------------------------------------------
            for t in range(NT):
                nc.sync.dma_start(out=out.ap()[t * P:(t + 1) * P, :],
                                  in_=ct[t][:])
    nc.compile()
    return nc


def _get_runner(replicas: int = 1):
    key = ("runner", replicas)
    if key not in _STATE:
        import jax
        import numpy as _np
        from jax.sharding import Mesh, PartitionSpec
        from jax.experimental.shard_map import shard_map
        import concourse.mybir as mybir
        from concourse import bass2jax
        from concourse.bass2jax import _bass_exec_p, install_neuronx_cc_hook

        install_neuronx_cc_hook()
        nc = build_nc(replicas)

        partition_name = (nc.partition_id_tensor.name
                          if nc.partition_id_tensor else None)
        in_names, out_names, out_avals, zero_shapes = [], [], [], []
        for alloc in nc.m.functions[0].allocations:
            if not isinstance(alloc, mybir.MemoryLocationSet):
                continue
            nm = alloc.memorylocations[0].name
            if alloc.kind == "ExternalInput":
                if nm != partition_name:
                    in_names.append(nm)
            elif alloc.kind == "ExternalOutput":
                out_names.append(nm)
                shape = tuple(alloc.tensor_shape)
                dtype = mybir.dt.np(alloc.dtype)
                out_avals.append(jax.core.ShapedArray(shape, dtype))
                zero_shapes.append((shape, dtype))
        all_in_names = list(in_names) + out_names
        if partition_name is not None:
            all_in_names.append(partition_name)

        def _body(*args):
            operands = list(args)
            if partition_name is not None:
                operands.append(bass2jax.partition_id_tensor())
            outs = _bass_exec_p.bind(
                *operands,
                out_avals=tuple(out_avals),
                in_names=tuple(all_in_names),
                out_names=tuple(out_names),
                lowering_input_output_aliases=(),
                sim_require_finite=True,
                sim_require_nnan=True,
                nc=nc,
            )
            return tuple(outs)

        devices = jax.devices()[:N_CORES]
        mesh = Mesh(_np.asarray(devices), ("core",))
        n_io = len(in_names) + len(out_names)
        fn = jax.jit(
            shard_map(_body, mesh=mesh,
                      in_specs=(PartitionSpec("core"),) * n_io,
                      out_specs=(PartitionSpec("core"),) * len(out_names),
                      check_rep=False),
            keep_unused=True,
        )
        _STATE[key] = (fn, in_names, out_names, out_avals, zero_shapes, mesh)
    return _STATE[key]


def _prepare(x, weight, bias, replicas=1):
    """device_put sharded inputs; returns a zero-arg callable running the
    kernel on device plus metadata for decoding outputs."""
    import jax
    from jax.sharding import NamedSharding, PartitionSpec
    fn, in_names, out_names, out_avals, zero_shapes, mesh = _get_runner(replicas)
    per_core = {
        "x": [x[c * M_LOC:(c + 1) * M_LOC] for c in range(N_CORES)],
        "weight": [weight] * N_CORES,
        "bias": [bias] * N_CORES,
    }
    concat_in = [np.concatenate(per_core[nm], axis=0) for nm in in_names]
    concat_zeros = [np.zeros((N_CORES * s[0], *s[1:]), d)
                    for (s, d) in zero_shapes]
    sharding = NamedSharding(mesh, PartitionSpec("core"))
    dev_in = [jax.device_put(a, sharding) for a in concat_in]
    dev_zero = [jax.device_put(a, sharding) for a in concat_zeros]

    def run():
        outs = fn(*dev_in, *dev_zero)
        jax.block_until_ready(outs)
        return outs

    return run, out_names, out_avals


def _run_sharded(x, weight, bias, replicas=1):
    run, out_names, out_avals = _prepare(x, weight, bias, replicas)
    return run(), out_names, out_avals


def kernel(x: np.ndarray, weight: np.ndarray, bias: np.ndarray) -> np.ndarray:
    x = np.ascontiguousarray(np.asarray(x, dtype=np.float32))
    weight = np.ascontiguousarray(np.asarray(weight, dtype=np.float32))
    bias = np.ascontiguousarray(np.asarray(bias, dtype=np.float32))
    outs, out_names, out_avals = _run_sharded(x, weight, bias, replicas=1)
    i = out_names.index("out")
    full = np.asarray(outs[i]).reshape(N_CORES * M_LOC, N)
    return full
